# revision 50
# baseline (speedup 1.0000x reference)
"""AttentionBlock kernel for 8 Trainium2 NeuronCores.

Sharding: core c -> batch b = c//2, parity p = c%2. Within a batch pair the
K/V projections are split by heads (core parity p computes K/V for global
heads 8p..8p+8 over the full context) and exchanged via pairwise AllGather
collectives; attention + FFN are split by query rows (parity-interleaved
128-row tiles, which balances causal attention work). All rank asymmetry
enters through input data (row shards, weight shards, masks) so one SPMD
program serves all 8 cores.

Math/dtypes:
- LN gains/biases are folded into the projection weights/biases host-side;
  all weights are host-rearranged so every DMA load is contiguous.
- Q/K/V projections and A@V run in fp8(e4m3) DoubleRow matmuls (2x rate,
  256-deep contraction per instruction) with per-tensor scaling.
- Logits are computed transposed (S^T = K Q^T) so the attention probs need
  no transpose before A@V; the softmax denominator comes from a ones-column
  appended to V that rides through the collective.
- QK^T in bf16; FFN in bf16; PSUM accumulation fp32 everywhere.
- Phase A (LN1) is chunk-interleaved with K/V production; attention is
  emission-interleaved with the Q projections; one 32KB tile serves as QT
  during attention and as h2T during the FFN.
"""
import sys
sys.path.insert(0, "/opt/trn_rl_repo")

from contextlib import ExitStack

import numpy as np
import ml_dtypes

import concourse.bacc as bacc
import concourse.bass as bass
import concourse.mybir as mybir
import concourse.tile as tile
from concourse import bass_utils

P = 128
F32 = mybir.dt.float32
BF16 = mybir.dt.bfloat16
FP8 = mybir.dt.float8e4

T, D, H, FF, HD = 2048, 2048, 16, 8192, 128
NT = T // P            # 16 context tiles
NQ = NT // 2           # 8 query slots per core
TQ = NQ * P            # 1024 query rows per core
ND = D // P            # 16 d tiles
NDP = ND // 2          # 8 d pairs
NF = FF // P           # 64 ffn hidden tiles
HL = H // 2            # 8 local heads per core
HG = 4                 # heads per V-production group
SCALE = 1.0 / np.sqrt(HD)
EPS = 1e-5

# fp8 scaling factors
S_H = 16.0             # ln1 output scale
S_W = 128.0            # qkv weight scale
S_P = 1.0 / 16.0       # attention prob scale (exp bias = ln S_P); max
                       # observed logit ~7.9 -> e^7.9/16 = 165 < fp8 max 240
S_V = 16.0             # v scale; ones column holds S_V so scales cancel

RG = [[0, 1], [2, 3], [4, 5], [6, 7]]
DR = mybir.MatmulPerfMode.DoubleRow
HEAD_ORDER = [0, 1, 2, 3, 8, 9, 10, 11, 4, 5, 6, 7, 12, 13, 14, 15]


def build_nc():
    nc = bacc.Bacc("TRN2", target_bir_lowering=False)

    x_ctx = nc.dram_tensor("x_ctx", [T, D], F32, kind="ExternalInput")
    xq = nc.dram_tensor("xq", [TQ, D], F32, kind="ExternalInput")
    # folded+scaled fp8 weights, host-arranged to the exact SBUF layout so
    # every load is contiguous: [.., dp, dpair, i, e]
    wq8 = nc.dram_tensor("wq8", [H, P, NDP, 2, HD], FP8, kind="ExternalInput")
    wk8 = nc.dram_tensor("wk8", [HL, P, NDP, 2, HD], FP8, kind="ExternalInput")
    wv8 = nc.dram_tensor("wv8", [2, P, NDP, 2, HG * HD], FP8,
                         kind="ExternalInput")
    bqf = nc.dram_tensor("bqf", [HD, H], F32, kind="ExternalInput")
    bkf = nc.dram_tensor("bkf", [HD, HL], F32, kind="ExternalInput")
    bvf = nc.dram_tensor("bvf", [HL, HD], F32, kind="ExternalInput")   # pre-scaled x2048
    W1 = nc.dram_tensor("W1", [NF, P, ND, P], BF16, kind="ExternalInput")
    b1 = nc.dram_tensor("b1", [P, NF], F32, kind="ExternalInput")      # be2-folded
    W2 = nc.dram_tensor("W2", [4, NF, P, 512], BF16, kind="ExternalInput")
    b2 = nc.dram_tensor("b2", [D], F32, kind="ExternalInput")
    maskT = nc.dram_tensor("maskT", [P, 2, P], F32, kind="ExternalInput")
    id8_in = nc.dram_tensor("id8_in", [P, P], FP8, kind="ExternalInput")
    idb_in = nc.dram_tensor("idb_in", [P, P], BF16, kind="ExternalInput")
    out = nc.dram_tensor("out", [TQ, D], F32, kind="ExternalOutput")

    ID = mybir.ActivationFunctionType.Identity
    EXP = mybir.ActivationFunctionType.Exp
    SQRT = mybir.ActivationFunctionType.Sqrt
    AL = mybir.AluOpType

    with tile.TileContext(nc) as tc:
        with tc.tile_pool(name="consts", bufs=1) as consts, \
             tc.tile_pool(name="dram", bufs=1, space="DRAM") as dpool:
            # identities from inputs (gpsimd affine_select is a slow Q7 launch)
            ident8 = consts.tile([P, P], FP8, tag="ident8")
            nc.sync.dma_start(out=ident8, in_=id8_in.ap())
            identb = consts.tile([P, P], BF16, tag="identb")
            nc.sync.dma_start(out=identb, in_=idb_in.ap())
            eps256 = consts.tile([P, 1], F32, tag="eps256")
            nc.gpsimd.memset(eps256, EPS / 256.0)
            eps_t = consts.tile([P, 1], F32, tag="epst")
            nc.gpsimd.memset(eps_t, EPS)
            lnsp = consts.tile([P, 1], F32, tag="lnsp")
            nc.gpsimd.memset(lnsp, float(np.log(S_P)))
            c_deq = consts.tile([P, 1], F32, tag="cdeq")
            nc.gpsimd.memset(c_deq, 1.0 / (S_H * S_W))
            negone = consts.tile([P, 1], F32, tag="negone")
            nc.gpsimd.memset(negone, -1.0)
            zero_t = consts.tile([P, 1], F32, tag="zt")
            nc.gpsimd.memset(zero_t, 0.0)
            # biases (host-transposed: straight contiguous loads)
            bqc = consts.tile([P, H], F32, tag="bqc")
            nc.sync.dma_start(out=bqc, in_=bqf.ap())
            bkc = consts.tile([P, HL], F32, tag="bkc")
            nc.sync.dma_start(out=bkc, in_=bkf.ap())
            bvb = [consts.tile([P, HG * HD], F32, tag=f"bvb{g}", name=f"bvb{g}")
                   for g in range(HL // HG)]
            for g in range(HL // HG):
                nc.sync.dma_start(out=bvb[g], in_=bass.AP(
                    tensor=bvf.ap().tensor, offset=g * HG * HD,
                    ap=[[0, P], [1, HG * HD]]))
            b1c = consts.tile([P, NF], F32, tag="b1c")
            nc.sync.dma_start(out=b1c, in_=b1.ap())
            b2b = consts.tile([P, D], F32, tag="b2b")
            nc.sync.dma_start(out=b2b, in_=bass.AP(
                tensor=b2.ap().tensor, offset=0, ap=[[0, P], [1, D]]))
            mskT = consts.tile([P, 2, P], F32, tag="mskT")
            nc.sync.dma_start(out=mskT, in_=maskT.ap())

            # DRAM: collective bounce buffers + x2 spill
            ccK_in = [dpool.tile([HG, P, T], BF16, tag=f"cKi{a}", name=f"cKi{a}")
                      for a in range(2)]
            ccK_out = [dpool.tile([2, HG, P, T], BF16, tag=f"cKo{a}", name=f"cKo{a}")
                       for a in range(2)]
            ccV_in = [dpool.tile([HG, P, NT, HD + 1], FP8, tag=f"cVi{a}",
                                 name=f"cVi{a}") for a in range(2)]
            ccV_out = [dpool.tile([2, HG, P, NT, HD + 1], FP8, tag=f"cVo{a}",
                                  name=f"cVo{a}") for a in range(2)]
            x2_dram = dpool.tile([TQ, D], F32, tag="x2d")

            # one 32KB tile serves as QT (through attention) then h2T (FFN)
            with tc.tile_pool(name="big", bufs=1) as bigp:
                QT = bigp.tile([P, 16, TQ], BF16, tag="big32")
                h2T = QT
                astack = ExitStack()
                atp = astack.enter_context(tc.tile_pool(name="attn", bufs=1))
                attn_sb = atp.tile([P, NQ, D], BF16, tag="attn_sb")

                # ====== A+B1+B2: LN1, K/V/Q, attention (interleaved) ======
                with tc.tile_pool(name="hT", bufs=1) as hTp, \
                     tc.tile_pool(name="phA", bufs=2) as pa, \
                     tc.tile_pool(name="phA2", bufs=2) as pa2, \
                     tc.tile_pool(name="phB1", bufs=2) as pb1:
                    hTs = [hTp.tile([P, 2, T + TQ], FP8, tag=f"hT{k}",
                                    name=f"hT{k}") for k in range(NDP)]

                    def ln1_tile(psA, src_ap, dst_col):
                        xin = pa.tile([P, D], F32, tag="xin")
                        nc.sync.dma_start(out=xin, in_=src_ap)
                        st = pa.tile([P, 4, 6], F32, tag="st")
                        xr = xin.rearrange("p (n f) -> p n f", n=4)
                        for s in range(4):
                            nc.vector.bn_stats(out=st[:, s, :], in_=xr[:, s, :])
                        mv = pa.tile([P, 2], F32, tag="mv")
                        nc.vector.bn_aggr(out=mv, in_=st)
                        # rstd16 = 16/sqrt(var+eps) = 1/sqrt((var+eps)/256)
                        rstd = pa.tile([P, 1], F32, tag="rstd")
                        nc.scalar.activation(out=rstd, in_=mv[:, 1:2],
                                             func=SQRT, bias=eps256,
                                             scale=1.0 / 256.0)
                        nc.vector.reciprocal(out=rstd, in_=rstd)
                        hb = pa2.tile([P, D], FP8, tag="hb")
                        nc.vector.tensor_scalar(
                            out=hb, in0=xin, scalar1=mv[:, 0:1],
                            scalar2=rstd, op0=AL.subtract, op1=AL.mult)
                        for d in range(ND):
                            # fp8 PE transpose needs output elem step 2
                            tp = psA.tile([P, P, 2], FP8, tag="tpA")
                            nc.tensor.transpose(
                                tp[:, :, 0], hb[:, d * P:(d + 1) * P], ident8)
                            dst = hTs[d // 2][:, d % 2, dst_col:dst_col + P]
                            if d % 2 == 0:
                                nc.scalar.activation(
                                    out=dst, in_=tp[:, :, 0], func=ID,
                                    bias=zero_t, scale=1.0)
                            else:
                                nc.vector.tensor_copy(dst, tp[:, :, 0])

                    def proj_psum(pp, wtile, cols, n):
                        # pp[e, n] += sum_d w[d,e] hT[d, cols:cols+n]
                        for k in range(NDP):
                            nc.tensor.matmul(
                                pp[:, :n], wtile[:, k, :, :],
                                hTs[k][:, :, cols:cols + n],
                                start=(k == 0), stop=(k == NDP - 1),
                                perf_mode=DR)

                    with tc.tile_pool(name="phB1w", bufs=1) as pbw, \
                         tc.tile_pool(name="psA", bufs=4, space="PSUM") as psA, \
                         tc.tile_pool(name="ps512", bufs=4, space="PSUM") as ps512:
                        # preload K/V weights (contiguous, gpsimd queue)
                        wk_ts = [pbw.tile([P, NDP, 2, HD], FP8, tag=f"wkt{h}",
                                          name=f"wkt{h}") for h in range(HL)]
                        for h in range(HL):
                            nc.gpsimd.dma_start(out=wk_ts[h], in_=wk8.ap()[h])
                        wv4s = [pbw.tile([P, NDP, 2, HG * HD], FP8,
                                         tag=f"wv4{a}", name=f"wv4{a}")
                                for a in range(2)]
                        for a in range(2):
                            nc.gpsimd.dma_start(out=wv4s[a], in_=wv8.ap()[a])
                        V4s = [pbw.tile([P, HG, NT, HD + 1], FP8, tag=f"V4{a}",
                                        name=f"V4{a}") for a in range(2)]
                        for a in range(2):
                            nc.gpsimd.memset(V4s[a][:, :, :, HD:HD + 1], S_V)

                        xr_ctx = x_ctx.ap().rearrange("(n p) d -> n p d", p=P)
                        for c in range(T // 512):   # ctx chunks of 512 rows
                            for g in range(4 * c, 4 * c + 4):
                                ln1_tile(psA, xr_ctx[g], g * P)
                            for hl in range(HL):
                                pk = ps512.tile([P, 512], F32, tag="p512")
                                proj_psum(pk, wk_ts[hl], c * 512, 512)
                                KTc = pb1.tile([P, 512], BF16, tag="KTc")
                                nc.vector.tensor_scalar(
                                    out=KTc, in0=pk, scalar1=c_deq,
                                    scalar2=bkc[:, hl:hl + 1],
                                    op0=AL.mult, op1=AL.add)
                                nc.gpsimd.dma_start(
                                    out=ccK_in[hl // HG][hl % HG, :,
                                                         c * 512:(c + 1) * 512],
                                    in_=KTc)
                            for a in range(2):
                                for s in range(4 * c, 4 * c + 4):
                                    pv = ps512.tile([P, HG * HD], F32, tag="p512")
                                    for k in range(NDP):
                                        nc.tensor.matmul(
                                            pv, hTs[k][:, :, s * P:(s + 1) * P],
                                            wv4s[a][:, k, :, :],
                                            start=(k == 0), stop=(k == NDP - 1),
                                            perf_mode=DR)
                                    nc.vector.tensor_tensor(
                                        out=pv, in0=pv, in1=bvb[a], op=AL.add)
                                    nc.scalar.activation(
                                        out=V4s[a][:, :, s, 0:HD], in_=pv,
                                        func=ID, bias=zero_t, scale=1.0 / S_W)
                        # ship V to the bounce and fire the exchange
                        for a in range(2):
                            for q in range(HG):
                                nc.gpsimd.dma_start(
                                    out=ccV_in[a][q],
                                    in_=V4s[a][:, q, :, :])
                            nc.gpsimd.collective_compute(
                                "AllGather", AL.bypass, replica_groups=RG,
                                ins=[ccK_in[a][:].opt()],
                                outs=[ccK_out[a][:].opt()])
                            nc.gpsimd.collective_compute(
                                "AllGather", AL.bypass, replica_groups=RG,
                                ins=[ccV_in[a][:].opt()],
                                outs=[ccV_out[a][:].opt()])
                        # LN for my query rows (hT q-region)
                        xr_q = xq.ap().rearrange("(n p) d -> n p d", p=P)
                        for j in range(NQ):
                            ln1_tile(psA, xr_q[j], T + j * P)

                    # pbw freed: B2 pools take its space. Q projections are
                    # emission-interleaved with attention, lagged by 2 heads.
                    with tc.tile_pool(name="phB2", bufs=2) as pb2, \
                         tc.tile_pool(name="phB2s", bufs=2) as pbs, \
                         tc.tile_pool(name="psQ", bufs=2, space="PSUM") as psQ, \
                         tc.tile_pool(name="psL", bufs=2, space="PSUM") as psL, \
                         tc.tile_pool(name="psAV", bufs=2, space="PSUM") as psAV:
                        def q_proj(h):
                            wq_t = pb1.tile([P, NDP, 2, HD], FP8, tag="wq")
                            nc.scalar.dma_start(out=wq_t, in_=wq8.ap()[h])
                            for c in range(TQ // 512):
                                pq = psQ.tile([P, 512], F32, tag="pq")
                                proj_psum(pq, wq_t, T + c * 512, 512)
                                nc.vector.tensor_scalar(
                                    out=QT[:, h, c * 512:(c + 1) * 512],
                                    in0=pq, scalar1=c_deq,
                                    scalar2=bqc[:, h:h + 1],
                                    op0=AL.mult, op1=AL.add)

                        def attn_head(hh):
                            a, chunk, idx = (0 if hh % 8 < 4 else 1,
                                             hh // 8, hh % 4)
                            KTh = pb2.tile([P, T], BF16, tag="KTh")
                            nc.sync.dma_start(
                                out=KTh, in_=ccK_out[a][chunk, idx])
                            Vh = pb2.tile([P, NQ, 2, HD + 1], FP8, tag="Vh")
                            nc.sync.dma_start(
                                out=Vh, in_=ccV_out[a][chunk, idx])
                            SPs = [pbs.tile([P, 2, (NQ - k) * P], FP8,
                                            tag=f"SP{k}", name=f"SP{k}")
                                   for k in range(NQ)]
                            for k in range(NQ):
                                w = (NQ - k) * P   # t-cols [k*128, 1024)
                                for i in range(2):
                                    s0 = (2 * k + i) * P
                                    lp = psL.tile([P, 1024], F32, tag="lp")
                                    for c in range((w + 511) // 512):
                                        cw = min(512, w - c * 512)
                                        nc.tensor.matmul(
                                            lp[:, c * 512:c * 512 + cw],
                                            KTh[:, s0:s0 + P],
                                            QT[:, hh, k * P + c * 512:
                                               k * P + c * 512 + cw],
                                            start=True, stop=True)
                                    nc.vector.tensor_tensor(
                                        out=lp[:, :P], in0=lp[:, :P],
                                        in1=mskT[:, i, :], op=AL.add)
                                    nc.scalar.activation(
                                        out=SPs[k][:, i, :], in_=lp[:, :w],
                                        func=EXP, bias=lnsp, scale=SCALE)
                            for j in range(NQ):
                                av = psAV.tile([P, HD + 1], F32, tag="av")
                                for k in range(j + 1):
                                    nc.tensor.matmul(
                                        av,
                                        SPs[k][:, :, (j - k) * P:(j - k + 1) * P],
                                        Vh[:, k, :, :],
                                        start=(k == 0), stop=(k == j),
                                        perf_mode=DR)
                                rs = pbs.tile([P, 1], F32, tag="rs")
                                nc.vector.reciprocal(out=rs, in_=av[:, HD:HD + 1])
                                nc.vector.tensor_scalar_mul(
                                    out=attn_sb[:, j, hh * HD:(hh + 1) * HD],
                                    in0=av[:, 0:HD], scalar1=rs)

                        LAG = 2
                        for i, hh in enumerate(HEAD_ORDER):
                            q_proj(hh)
                            if i >= LAG:
                                attn_head(HEAD_ORDER[i - LAG])
                        for i in range(H - LAG, H):
                            attn_head(HEAD_ORDER[i])

                # ============ Phase C: residual + LN2 -> h2T ============
                with tc.tile_pool(name="phC", bufs=3) as pc, \
                     tc.tile_pool(name="phC2", bufs=2) as pc2, \
                     tc.tile_pool(name="psC", bufs=4, space="PSUM") as psC:
                    xr_q = xq.ap().rearrange("(n p) d -> n p d", p=P)
                    for t in range(NQ):
                        xt = pc.tile([P, D], F32, tag="xt")
                        nc.sync.dma_start(out=xt, in_=xr_q[t])
                        x2 = pc.tile([P, D], F32, tag="x2t")
                        nc.vector.tensor_tensor(
                            out=x2, in0=xt, in1=attn_sb[:, t, :], op=AL.add)
                        nc.sync.dma_start(
                            out=x2_dram[t * P:(t + 1) * P, :], in_=x2)
                        st = pc.tile([P, 4, 6], F32, tag="st2")
                        x2r = x2.rearrange("p (n f) -> p n f", n=4)
                        for s in range(4):
                            nc.vector.bn_stats(out=st[:, s, :], in_=x2r[:, s, :])
                        mv = pc.tile([P, 2], F32, tag="mv2")
                        nc.vector.bn_aggr(out=mv, in_=st)
                        rstd = pc.tile([P, 1], F32, tag="rstd2")
                        nc.scalar.activation(out=rstd, in_=mv[:, 1:2],
                                             func=SQRT, bias=eps_t, scale=1.0)
                        nc.vector.reciprocal(out=rstd, in_=rstd)
                        h2 = pc2.tile([P, D], BF16, tag="h2tmp")
                        nc.vector.tensor_scalar(
                            out=h2, in0=x2, scalar1=mv[:, 0:1],
                            scalar2=rstd, op0=AL.subtract, op1=AL.mult)
                        for d in range(ND):
                            tp = psC.tile([P, P], BF16, tag="tpC")
                            nc.tensor.transpose(
                                tp, h2[:, d * P:(d + 1) * P], identb)
                            if d % 2 == 0:
                                nc.scalar.activation(
                                    out=h2T[:, d, t * P:(t + 1) * P],
                                    in_=tp, func=ID, bias=zero_t, scale=1.0)
                            else:
                                nc.vector.tensor_copy(
                                    h2T[:, d, t * P:(t + 1) * P], tp)

                # ============ FFN (attn pool freed; h2T = recycled QT) ======
                astack.close()
                with tc.tile_pool(name="us", bufs=1) as usp, \
                     tc.tile_pool(name="phW1", bufs=2) as pw1, \
                     tc.tile_pool(name="phW2", bufs=3) as pw2, \
                     tc.tile_pool(name="phE", bufs=1) as pe:
                    Us = usp.tile([P, NF, TQ], BF16, tag="Us")
                    with tc.tile_pool(name="psU", bufs=4, space="PSUM") as psU:
                        for f in range(NF):
                            w1f = pw1.tile([P, ND, P], BF16, tag="w1f")
                            nc.scalar.dma_start(out=w1f, in_=W1.ap()[f])
                            for c in range(TQ // 512):
                                up = psU.tile([P, 512], F32, tag="up")
                                for d in range(ND):
                                    nc.tensor.matmul(
                                        up, w1f[:, d, :],
                                        h2T[:, d, c * 512:(c + 1) * 512],
                                        start=(d == 0), stop=(d == ND - 1))
                                nc.vector.tensor_scalar(
                                    out=Us[:, f, c * 512:(c + 1) * 512],
                                    in0=up, scalar1=b1c[:, f:f + 1],
                                    scalar2=zero_t, op0=AL.add, op1=AL.max)
                    with tc.tile_pool(name="psO", bufs=1, space="PSUM") as psO:
                        for db in range(D // 512):
                            ops = [psO.tile([P, 512], F32, tag=f"op{t}",
                                            name=f"op{t}") for t in range(NQ)]
                            # precompute x2+b2 while the f-loop matmuls run,
                            # so each psum bank drains with one vector op
                            xbs = pe.tile([P, NQ, 512], F32, tag="xbs")
                            for t in range(NQ):
                                nc.sync.dma_start(
                                    out=xbs[:, t, :],
                                    in_=x2_dram[t * P:(t + 1) * P,
                                                db * 512:(db + 1) * 512])
                                nc.vector.tensor_tensor(
                                    out=xbs[:, t, :], in0=xbs[:, t, :],
                                    in1=b2b[:, db * 512:(db + 1) * 512],
                                    op=AL.add)
                            for f in range(NF):
                                w2t = pw2.tile([P, 512], BF16, tag="w2t")
                                nc.sync.dma_start(out=w2t, in_=W2.ap()[db, f])
                                for t in range(NQ):
                                    nc.tensor.matmul(
                                        ops[t], Us[:, f, t * P:(t + 1) * P], w2t,
                                        start=(f == 0), stop=(f == NF - 1))
                            for t in range(NQ):
                                nc.vector.tensor_tensor(
                                    out=xbs[:, t, :], in0=ops[t],
                                    in1=xbs[:, t, :], op=AL.add)
                                nc.sync.dma_start(
                                    out=out.ap()[t * P:(t + 1) * P,
                                                 db * 512:(db + 1) * 512],
                                    in_=xbs[:, t, :])

    nc.compile()
    return nc


_NC_CACHE = {}


def get_nc():
    if "full" not in _NC_CACHE:
        _NC_CACHE["full"] = build_nc()
    return _NC_CACHE["full"]


def make_in_maps(inputs):
    f32 = np.float32
    fp8 = ml_dtypes.float8_e4m3
    bf = ml_dtypes.bfloat16
    x = np.asarray(inputs["x"], f32)
    B = x.shape[0]
    Wq = np.asarray(inputs["Wq"], f32)
    Wk = np.asarray(inputs["Wk"], f32)
    Wv = np.asarray(inputs["Wv"], f32)
    bq = np.asarray(inputs["bq"], f32)
    bk = np.asarray(inputs["bk"], f32)
    bv = np.asarray(inputs["bv"], f32)
    g1 = np.asarray(inputs["g1"], f32)
    be1 = np.asarray(inputs["be1"], f32)
    g2 = np.asarray(inputs["g2"], f32)
    be2 = np.asarray(inputs["be2"], f32)
    W1 = np.asarray(inputs["W1"], f32)
    b1 = np.asarray(inputs["b1"], f32)
    W2 = np.asarray(inputs["W2"], f32)
    b2 = np.asarray(inputs["b2"], f32)

    # fold LN1 gains into qkv weights, biases into qkv biases
    wq8 = (g1[None, :, None] * Wq * S_W).astype(fp8)
    wk8 = (g1[None, :, None] * Wk * S_W).astype(fp8)
    wv8 = (g1[None, :, None] * Wv * S_W).astype(fp8)
    bqf = (bq + np.einsum("d,hde->he", be1, Wq)).astype(f32)
    bkf = (bk + np.einsum("d,hde->he", be1, Wk)).astype(f32)
    bvf = ((bv + np.einsum("d,hde->he", be1, Wv)) * (S_H * S_W)).astype(f32)
    # fold LN2 gains into W1
    W1f = (g2[:, None] * W1).astype(bf)
    b1f = (b1 + be2 @ W1).astype(f32)
    W2b = W2.astype(bf)

    def arr_qkv(w):
        # [h, 256k+128i+dp, e] -> [h, dp, k, i, e]
        n = w.shape[0]
        return np.ascontiguousarray(
            w.reshape(n, NDP, 2, P, HD).transpose(0, 3, 1, 2, 4))

    shared = {
        "wq8": arr_qkv(wq8),
        "bqf": np.ascontiguousarray(bqf.T),
        # W1 [d*128+dp, f*128+c] -> [f, dp, d, c]
        "W1": np.ascontiguousarray(
            W1f.reshape(ND, P, NF, P).transpose(2, 1, 0, 3)),
        "b1": np.ascontiguousarray(b1f.reshape(NF, P).T),
        # W2 [f*128+fp, db*512+c] -> [db, f, fp, c]
        "W2": np.ascontiguousarray(
            W2b.reshape(NF, P, 4, 512).transpose(2, 0, 1, 3)),
        "b2": b2,
        "id8_in": np.eye(P, dtype=np.float32).astype(fp8),
        "idb_in": np.eye(P, dtype=np.float32).astype(bf),
    }
    in_maps = []
    for c in range(2 * B):
        b, p = c // 2, c % 2
        rows = np.concatenate([np.arange(g * P, (g + 1) * P)
                               for g in range(p, NT, 2)])
        # maskT[sp, i, tp]: -1e9 where (i-p)*128 + sp > tp
        sp = np.arange(P)[:, None, None]
        i = np.arange(2)[None, :, None]
        tp = np.arange(P)[None, None, :]
        m = np.where((i - p) * P + sp > tp, f32(-1e9), f32(0.0))
        im = dict(shared)
        im["x_ctx"] = x[b]
        im["xq"] = x[b][rows]
        im["wk8"] = arr_qkv(wk8[HL * p:HL * (p + 1)])
        # wv: [a, dp, k, i, q*HD+e] for local head groups of 4
        wvs = wv8[HL * p:HL * (p + 1)]
        im["wv8"] = np.ascontiguousarray(
            wvs.reshape(2, HG, NDP, 2, P, HD)
               .transpose(0, 4, 2, 3, 1, 5).reshape(2, P, NDP, 2, HG * HD))
        im["bkf"] = np.ascontiguousarray(bkf[HL * p:HL * (p + 1)].T)
        im["bvf"] = bvf[HL * p:HL * (p + 1)]
        im["maskT"] = np.ascontiguousarray(m)
        in_maps.append(im)
    return in_maps


def assemble(results, B):
    out = np.zeros((B, T, D), np.float32)
    for c in range(2 * B):
        b, p = c // 2, c % 2
        rows = np.concatenate([np.arange(g * P, (g + 1) * P)
                               for g in range(p, NT, 2)])
        out[b][rows] = results[c]["out"]
    return out


def run(inputs, trace=False, **kw):
    nc = get_nc()
    in_maps = make_in_maps(inputs)
    res = bass_utils.run_bass_kernel_spmd(
        nc, in_maps, core_ids=list(range(len(in_maps))), trace=trace, **kw)
    B = np.asarray(inputs["x"]).shape[0]
    return assemble(res.results, B), res


def kernel(**inputs):
    out, _ = run(inputs)
    return out


# revision 52
# speedup vs baseline: 1.2240x; 1.2240x over previous
"""AttentionBlock kernel for 8 Trainium2 NeuronCores.

Sharding: core c -> batch b = c//2, parity p = c%2. Within a batch pair the
K/V projections are split by heads (core parity p computes K/V for global
heads 8p..8p+8 over the full context) and exchanged via pairwise AllGather
collectives; attention + FFN are split by query rows (parity-interleaved
128-row tiles, which balances causal attention work). All rank asymmetry
enters through input data (row shards, weight shards, masks) so one SPMD
program serves all 8 cores.

Math/dtypes:
- LN gains/biases are folded into the projection weights/biases host-side;
  all weights are host-rearranged so every DMA load is contiguous.
- Q/K/V projections and A@V run in fp8(e4m3) DoubleRow matmuls (2x rate,
  256-deep contraction per instruction) with per-tensor scaling.
- Logits are computed transposed (S^T = K Q^T) so the attention probs need
  no transpose before A@V; the softmax denominator comes from a ones-column
  appended to V that rides through the collective.
- QK^T in bf16; FFN in bf16; PSUM accumulation fp32 everywhere.
- Phase A (LN1) is chunk-interleaved with K/V production; attention is
  emission-interleaved with the Q projections; one 32KB tile serves as QT
  during attention and as h2T during the FFN.
"""
import sys
sys.path.insert(0, "/opt/trn_rl_repo")

from contextlib import ExitStack

import numpy as np
import ml_dtypes

import concourse.bacc as bacc
import concourse.bass as bass
import concourse.mybir as mybir
import concourse.tile as tile
from concourse import bass_utils

P = 128
F32 = mybir.dt.float32
BF16 = mybir.dt.bfloat16
FP8 = mybir.dt.float8e4

T, D, H, FF, HD = 2048, 2048, 16, 8192, 128
NT = T // P            # 16 context tiles
NQ = NT // 2           # 8 query slots per core
TQ = NQ * P            # 1024 query rows per core
ND = D // P            # 16 d tiles
NDP = ND // 2          # 8 d pairs
NF = FF // P           # 64 ffn hidden tiles
HL = H // 2            # 8 local heads per core
HG = 4                 # heads per V-production group
SCALE = 1.0 / np.sqrt(HD)
EPS = 1e-5

# fp8 scaling factors
S_H = 16.0             # ln1 output scale
S_W = 128.0            # qkv weight scale
S_P = 1.0 / 16.0       # attention prob scale (exp bias = ln S_P); max
                       # observed logit ~7.9 -> e^7.9/16 = 165 < fp8 max 240
S_V = 16.0             # v scale; ones column holds S_V so scales cancel

RG = [[0, 1], [2, 3], [4, 5], [6, 7]]
DR = mybir.MatmulPerfMode.DoubleRow
HEAD_ORDER = [0, 1, 2, 3, 8, 9, 10, 11, 4, 5, 6, 7, 12, 13, 14, 15]


def build_nc():
    nc = bacc.Bacc("TRN2", target_bir_lowering=False)

    x_ctx = nc.dram_tensor("x_ctx", [T, D], F32, kind="ExternalInput")
    xq = nc.dram_tensor("xq", [TQ, D], F32, kind="ExternalInput")
    # folded+scaled fp8 weights, host-arranged to the exact SBUF layout so
    # every load is contiguous: [.., dp, dpair, i, e]
    wq8 = nc.dram_tensor("wq8", [H, P, NDP, 2, HD], FP8, kind="ExternalInput")
    wk8 = nc.dram_tensor("wk8", [HL, P, NDP, 2, HD], FP8, kind="ExternalInput")
    wv8 = nc.dram_tensor("wv8", [2, P, NDP, 2, HG * HD], FP8,
                         kind="ExternalInput")
    bqf = nc.dram_tensor("bqf", [HD, H], F32, kind="ExternalInput")
    bkf = nc.dram_tensor("bkf", [HD, HL], F32, kind="ExternalInput")
    bvf = nc.dram_tensor("bvf", [HL, HD], F32, kind="ExternalInput")   # pre-scaled x2048
    W1 = nc.dram_tensor("W1", [NF, P, ND, P], BF16, kind="ExternalInput")
    b1 = nc.dram_tensor("b1", [P, NF], F32, kind="ExternalInput")      # be2-folded
    W2 = nc.dram_tensor("W2", [4, NF, P, 512], BF16, kind="ExternalInput")
    b2 = nc.dram_tensor("b2", [D], F32, kind="ExternalInput")
    maskT = nc.dram_tensor("maskT", [P, 2, P], F32, kind="ExternalInput")
    id8_in = nc.dram_tensor("id8_in", [P, P], FP8, kind="ExternalInput")
    idb_in = nc.dram_tensor("idb_in", [P, P], BF16, kind="ExternalInput")
    out = nc.dram_tensor("out", [TQ, D], F32, kind="ExternalOutput")

    ID = mybir.ActivationFunctionType.Identity
    EXP = mybir.ActivationFunctionType.Exp
    SQRT = mybir.ActivationFunctionType.Sqrt
    AL = mybir.AluOpType

    with tile.TileContext(nc) as tc:
        with tc.tile_pool(name="consts", bufs=1) as consts, \
             tc.tile_pool(name="dram", bufs=1, space="DRAM") as dpool:
            # identities from inputs (gpsimd affine_select is a slow Q7 launch)
            ident8 = consts.tile([P, P], FP8, tag="ident8")
            nc.sync.dma_start(out=ident8, in_=id8_in.ap())
            identb = consts.tile([P, P], BF16, tag="identb")
            nc.sync.dma_start(out=identb, in_=idb_in.ap())
            eps256 = consts.tile([P, 1], F32, tag="eps256")
            nc.gpsimd.memset(eps256, EPS / 256.0)
            eps_t = consts.tile([P, 1], F32, tag="epst")
            nc.gpsimd.memset(eps_t, EPS)
            lnsp = consts.tile([P, 1], F32, tag="lnsp")
            nc.gpsimd.memset(lnsp, float(np.log(S_P)))
            c_deq = consts.tile([P, 1], F32, tag="cdeq")
            nc.gpsimd.memset(c_deq, 1.0 / (S_H * S_W))
            negone = consts.tile([P, 1], F32, tag="negone")
            nc.gpsimd.memset(negone, -1.0)
            zero_t = consts.tile([P, 1], F32, tag="zt")
            nc.gpsimd.memset(zero_t, 0.0)
            # biases (host-transposed: straight contiguous loads)
            bqc = consts.tile([P, H], F32, tag="bqc")
            nc.sync.dma_start(out=bqc, in_=bqf.ap())
            bkc = consts.tile([P, HL], F32, tag="bkc")
            nc.sync.dma_start(out=bkc, in_=bkf.ap())
            bvb = [consts.tile([P, HG * HD], F32, tag=f"bvb{g}", name=f"bvb{g}")
                   for g in range(HL // HG)]
            for g in range(HL // HG):
                nc.sync.dma_start(out=bvb[g], in_=bass.AP(
                    tensor=bvf.ap().tensor, offset=g * HG * HD,
                    ap=[[0, P], [1, HG * HD]]))
            b1c = consts.tile([P, NF], F32, tag="b1c")
            nc.sync.dma_start(out=b1c, in_=b1.ap())
            b2b = consts.tile([P, D], F32, tag="b2b")
            nc.sync.dma_start(out=b2b, in_=bass.AP(
                tensor=b2.ap().tensor, offset=0, ap=[[0, P], [1, D]]))
            mskT = consts.tile([P, 2, P], F32, tag="mskT")
            nc.sync.dma_start(out=mskT, in_=maskT.ap())

            # DRAM: collective bounce buffers + x2 spill
            ccK_in = [dpool.tile([HG, P, T], BF16, tag=f"cKi{a}", name=f"cKi{a}")
                      for a in range(2)]
            ccK_out = [dpool.tile([2, HG, P, T], BF16, tag=f"cKo{a}", name=f"cKo{a}")
                       for a in range(2)]
            ccV_in = [dpool.tile([HG, P, NT, HD + 1], FP8, tag=f"cVi{a}",
                                 name=f"cVi{a}") for a in range(2)]
            ccV_out = [dpool.tile([2, HG, P, NT, HD + 1], FP8, tag=f"cVo{a}",
                                  name=f"cVo{a}") for a in range(2)]
            x2_dram = dpool.tile([TQ, D], F32, tag="x2d")

            # one 32KB tile serves as QT (through attention) then h2T (FFN)
            with tc.tile_pool(name="big", bufs=1) as bigp:
                QT = bigp.tile([P, 16, TQ], BF16, tag="big32")
                h2T = QT
                astack = ExitStack()
                atp = astack.enter_context(tc.tile_pool(name="attn", bufs=1))
                attn_sb = atp.tile([P, NQ, D], BF16, tag="attn_sb")

                # ====== A+B1+B2: LN1, K/V/Q, attention (interleaved) ======
                with tc.tile_pool(name="hT", bufs=1) as hTp, \
                     tc.tile_pool(name="phA", bufs=2) as pa, \
                     tc.tile_pool(name="phA2", bufs=2) as pa2, \
                     tc.tile_pool(name="phB1", bufs=2) as pb1:
                    hTs = [hTp.tile([P, 2, T + TQ], FP8, tag=f"hT{k}",
                                    name=f"hT{k}") for k in range(NDP)]

                    def ln1_tile(psA, src_ap, dst_col):
                        xin = pa.tile([P, D], F32, tag="xin")
                        nc.sync.dma_start(out=xin, in_=src_ap)
                        st = pa.tile([P, 4, 6], F32, tag="st")
                        xr = xin.rearrange("p (n f) -> p n f", n=4)
                        for s in range(4):
                            nc.vector.bn_stats(out=st[:, s, :], in_=xr[:, s, :])
                        mv = pa.tile([P, 2], F32, tag="mv")
                        nc.vector.bn_aggr(out=mv, in_=st)
                        # rstd16 = 16/sqrt(var+eps) = 1/sqrt((var+eps)/256)
                        rstd = pa.tile([P, 1], F32, tag="rstd")
                        nc.scalar.activation(out=rstd, in_=mv[:, 1:2],
                                             func=SQRT, bias=eps256,
                                             scale=1.0 / 256.0)
                        nc.vector.reciprocal(out=rstd, in_=rstd)
                        hb = pa2.tile([P, D], FP8, tag="hb")
                        nc.vector.tensor_scalar(
                            out=hb, in0=xin, scalar1=mv[:, 0:1],
                            scalar2=rstd, op0=AL.subtract, op1=AL.mult)
                        for d in range(ND):
                            # fp8 PE transpose needs output elem step 2
                            tp = psA.tile([P, P, 2], FP8, tag="tpA")
                            nc.tensor.transpose(
                                tp[:, :, 0], hb[:, d * P:(d + 1) * P], ident8)
                            dst = hTs[d // 2][:, d % 2, dst_col:dst_col + P]
                            if d % 2 == 0:
                                nc.scalar.activation(
                                    out=dst, in_=tp[:, :, 0], func=ID,
                                    bias=zero_t, scale=1.0)
                            else:
                                nc.vector.tensor_copy(dst, tp[:, :, 0])

                    def proj_psum(pp, wtile, cols, n):
                        # pp[e, n] += sum_d w[d,e] hT[d, cols:cols+n]
                        for k in range(NDP):
                            nc.tensor.matmul(
                                pp[:, :n], wtile[:, k, :, :],
                                hTs[k][:, :, cols:cols + n],
                                start=(k == 0), stop=(k == NDP - 1),
                                perf_mode=DR)

                    with tc.tile_pool(name="phB1w", bufs=1) as pbw, \
                         tc.tile_pool(name="psA", bufs=4, space="PSUM") as psA, \
                         tc.tile_pool(name="ps512", bufs=4, space="PSUM") as ps512:
                        # preload K/V weights (contiguous, gpsimd queue)
                        wk_ts = [pbw.tile([P, NDP, 2, HD], FP8, tag=f"wkt{h}",
                                          name=f"wkt{h}") for h in range(HL)]
                        for h in range(HL):
                            nc.gpsimd.dma_start(out=wk_ts[h], in_=wk8.ap()[h])
                        wv4s = [pbw.tile([P, NDP, 2, HG * HD], FP8,
                                         tag=f"wv4{a}", name=f"wv4{a}")
                                for a in range(2)]
                        for a in range(2):
                            nc.gpsimd.dma_start(out=wv4s[a], in_=wv8.ap()[a])
                        V4s = [pbw.tile([P, HG, NT, HD + 1], FP8, tag=f"V4{a}",
                                        name=f"V4{a}") for a in range(2)]
                        for a in range(2):
                            nc.gpsimd.memset(V4s[a][:, :, :, HD:HD + 1], S_V)

                        xr_ctx = x_ctx.ap().rearrange("(n p) d -> n p d", p=P)
                        for c in range(T // 512):   # ctx chunks of 512 rows
                            for g in range(4 * c, 4 * c + 4):
                                ln1_tile(psA, xr_ctx[g], g * P)
                            for hl in range(HL):
                                pk = ps512.tile([P, 512], F32, tag="p512")
                                proj_psum(pk, wk_ts[hl], c * 512, 512)
                                KTc = pb1.tile([P, 512], BF16, tag="KTc")
                                nc.vector.tensor_scalar(
                                    out=KTc, in0=pk, scalar1=c_deq,
                                    scalar2=bkc[:, hl:hl + 1],
                                    op0=AL.mult, op1=AL.add)
                                nc.gpsimd.dma_start(
                                    out=ccK_in[hl // HG][hl % HG, :,
                                                         c * 512:(c + 1) * 512],
                                    in_=KTc)
                            for a in range(2):
                                for s in range(4 * c, 4 * c + 4):
                                    pv = ps512.tile([P, HG * HD], F32, tag="p512")
                                    for k in range(NDP):
                                        nc.tensor.matmul(
                                            pv, hTs[k][:, :, s * P:(s + 1) * P],
                                            wv4s[a][:, k, :, :],
                                            start=(k == 0), stop=(k == NDP - 1),
                                            perf_mode=DR)
                                    nc.vector.tensor_tensor(
                                        out=pv, in0=pv, in1=bvb[a], op=AL.add)
                                    nc.scalar.activation(
                                        out=V4s[a][:, :, s, 0:HD], in_=pv,
                                        func=ID, bias=zero_t, scale=1.0 / S_W)
                        # ship V to the bounce and fire the exchange
                        for a in range(2):
                            for q in range(HG):
                                nc.gpsimd.dma_start(
                                    out=ccV_in[a][q],
                                    in_=V4s[a][:, q, :, :])
                            nc.gpsimd.collective_compute(
                                "AllGather", AL.bypass, replica_groups=RG,
                                ins=[ccK_in[a][:].opt()],
                                outs=[ccK_out[a][:].opt()])
                            nc.gpsimd.collective_compute(
                                "AllGather", AL.bypass, replica_groups=RG,
                                ins=[ccV_in[a][:].opt()],
                                outs=[ccV_out[a][:].opt()])
                        # LN for my query rows (hT q-region)
                        xr_q = xq.ap().rearrange("(n p) d -> n p d", p=P)
                        for j in range(NQ):
                            ln1_tile(psA, xr_q[j], T + j * P)
                        # Q for all 16 global heads over my TQ rows
                        for h in HEAD_ORDER:
                            wq_t = pb1.tile([P, NDP, 2, HD], FP8, tag="wq")
                            nc.scalar.dma_start(out=wq_t, in_=wq8.ap()[h])
                            for c in range(TQ // 512):
                                pq = ps512.tile([P, 512], F32, tag="p512")
                                proj_psum(pq, wq_t, T + c * 512, 512)
                                nc.vector.tensor_scalar(
                                    out=QT[:, h, c * 512:(c + 1) * 512],
                                    in0=pq, scalar1=c_deq,
                                    scalar2=bqc[:, h:h + 1],
                                    op0=AL.mult, op1=AL.add)

                    # pbw freed: B2 pools take its space
                    with tc.tile_pool(name="phB2", bufs=2) as pb2, \
                         tc.tile_pool(name="phB2s", bufs=2) as pbs, \
                         tc.tile_pool(name="psL", bufs=3, space="PSUM") as psL, \
                         tc.tile_pool(name="psAV", bufs=2, space="PSUM") as psAV:
                        def attn_head(hh):
                            a, chunk, idx = (0 if hh % 8 < 4 else 1,
                                             hh // 8, hh % 4)
                            KTh = pb2.tile([P, T], BF16, tag="KTh")
                            nc.sync.dma_start(
                                out=KTh, in_=ccK_out[a][chunk, idx])
                            Vh = pb2.tile([P, NQ, 2, HD + 1], FP8, tag="Vh")
                            nc.sync.dma_start(
                                out=Vh, in_=ccV_out[a][chunk, idx])
                            SPs = [pbs.tile([P, 2, (NQ - k) * P], FP8,
                                            tag=f"SP{k}", name=f"SP{k}")
                                   for k in range(NQ)]
                            for k in range(NQ):
                                w = (NQ - k) * P   # t-cols [k*128, 1024)
                                for i in range(2):
                                    s0 = (2 * k + i) * P
                                    lp = psL.tile([P, 1024], F32, tag="lp")
                                    for c in range((w + 511) // 512):
                                        cw = min(512, w - c * 512)
                                        nc.tensor.matmul(
                                            lp[:, c * 512:c * 512 + cw],
                                            KTh[:, s0:s0 + P],
                                            QT[:, hh, k * P + c * 512:
                                               k * P + c * 512 + cw],
                                            start=True, stop=True)
                                    nc.vector.tensor_tensor(
                                        out=lp[:, :P], in0=lp[:, :P],
                                        in1=mskT[:, i, :], op=AL.add)
                                    nc.scalar.activation(
                                        out=SPs[k][:, i, :], in_=lp[:, :w],
                                        func=EXP, bias=lnsp, scale=SCALE)
                            for j in range(NQ):
                                av = psAV.tile([P, HD + 1], F32, tag="av")
                                for k in range(j + 1):
                                    nc.tensor.matmul(
                                        av,
                                        SPs[k][:, :, (j - k) * P:(j - k + 1) * P],
                                        Vh[:, k, :, :],
                                        start=(k == 0), stop=(k == j),
                                        perf_mode=DR)
                                rs = pbs.tile([P, 1], F32, tag="rs")
                                nc.vector.reciprocal(out=rs, in_=av[:, HD:HD + 1])
                                nc.vector.tensor_scalar_mul(
                                    out=attn_sb[:, j, hh * HD:(hh + 1) * HD],
                                    in0=av[:, 0:HD], scalar1=rs)

                        for hh in HEAD_ORDER:
                            attn_head(hh)

                # ============ Phase C: residual + LN2 -> h2T ============
                with tc.tile_pool(name="phC", bufs=3) as pc, \
                     tc.tile_pool(name="phC2", bufs=2) as pc2, \
                     tc.tile_pool(name="psC", bufs=4, space="PSUM") as psC:
                    xr_q = xq.ap().rearrange("(n p) d -> n p d", p=P)
                    for t in range(NQ):
                        xt = pc.tile([P, D], F32, tag="xt")
                        nc.sync.dma_start(out=xt, in_=xr_q[t])
                        x2 = pc.tile([P, D], F32, tag="x2t")
                        nc.vector.tensor_tensor(
                            out=x2, in0=xt, in1=attn_sb[:, t, :], op=AL.add)
                        nc.sync.dma_start(
                            out=x2_dram[t * P:(t + 1) * P, :], in_=x2)
                        st = pc.tile([P, 4, 6], F32, tag="st2")
                        x2r = x2.rearrange("p (n f) -> p n f", n=4)
                        for s in range(4):
                            nc.vector.bn_stats(out=st[:, s, :], in_=x2r[:, s, :])
                        mv = pc.tile([P, 2], F32, tag="mv2")
                        nc.vector.bn_aggr(out=mv, in_=st)
                        rstd = pc.tile([P, 1], F32, tag="rstd2")
                        nc.scalar.activation(out=rstd, in_=mv[:, 1:2],
                                             func=SQRT, bias=eps_t, scale=1.0)
                        nc.vector.reciprocal(out=rstd, in_=rstd)
                        h2 = pc2.tile([P, D], BF16, tag="h2tmp")
                        nc.vector.tensor_scalar(
                            out=h2, in0=x2, scalar1=mv[:, 0:1],
                            scalar2=rstd, op0=AL.subtract, op1=AL.mult)
                        for d in range(ND):
                            tp = psC.tile([P, P], BF16, tag="tpC")
                            nc.tensor.transpose(
                                tp, h2[:, d * P:(d + 1) * P], identb)
                            if d % 2 == 0:
                                nc.scalar.activation(
                                    out=h2T[:, d, t * P:(t + 1) * P],
                                    in_=tp, func=ID, bias=zero_t, scale=1.0)
                            else:
                                nc.vector.tensor_copy(
                                    h2T[:, d, t * P:(t + 1) * P], tp)

                # ============ FFN (attn pool freed; h2T = recycled QT) ======
                astack.close()
                with tc.tile_pool(name="us", bufs=1) as usp, \
                     tc.tile_pool(name="phW1", bufs=2) as pw1, \
                     tc.tile_pool(name="phW2", bufs=3) as pw2, \
                     tc.tile_pool(name="phE", bufs=1) as pe:
                    Us = usp.tile([P, NF, TQ], BF16, tag="Us")
                    with tc.tile_pool(name="psU", bufs=4, space="PSUM") as psU:
                        for f in range(NF):
                            w1f = pw1.tile([P, ND, P], BF16, tag="w1f")
                            nc.scalar.dma_start(out=w1f, in_=W1.ap()[f])
                            for c in range(TQ // 512):
                                up = psU.tile([P, 512], F32, tag="up")
                                for d in range(ND):
                                    nc.tensor.matmul(
                                        up, w1f[:, d, :],
                                        h2T[:, d, c * 512:(c + 1) * 512],
                                        start=(d == 0), stop=(d == ND - 1))
                                nc.vector.tensor_scalar(
                                    out=Us[:, f, c * 512:(c + 1) * 512],
                                    in0=up, scalar1=b1c[:, f:f + 1],
                                    scalar2=zero_t, op0=AL.add, op1=AL.max)
                    with tc.tile_pool(name="psO", bufs=1, space="PSUM") as psO:
                        for db in range(D // 512):
                            ops = [psO.tile([P, 512], F32, tag=f"op{t}",
                                            name=f"op{t}") for t in range(NQ)]
                            # precompute x2+b2 while the f-loop matmuls run,
                            # so each psum bank drains with one vector op
                            xbs = pe.tile([P, NQ, 512], F32, tag="xbs")
                            for t in range(NQ):
                                nc.sync.dma_start(
                                    out=xbs[:, t, :],
                                    in_=x2_dram[t * P:(t + 1) * P,
                                                db * 512:(db + 1) * 512])
                                nc.vector.tensor_tensor(
                                    out=xbs[:, t, :], in0=xbs[:, t, :],
                                    in1=b2b[:, db * 512:(db + 1) * 512],
                                    op=AL.add)
                            for f in range(NF):
                                w2t = pw2.tile([P, 512], BF16, tag="w2t")
                                nc.sync.dma_start(out=w2t, in_=W2.ap()[db, f])
                                for t in range(NQ):
                                    nc.tensor.matmul(
                                        ops[t], Us[:, f, t * P:(t + 1) * P], w2t,
                                        start=(f == 0), stop=(f == NF - 1))
                            for t in range(NQ):
                                nc.vector.tensor_tensor(
                                    out=xbs[:, t, :], in0=ops[t],
                                    in1=xbs[:, t, :], op=AL.add)
                                nc.sync.dma_start(
                                    out=out.ap()[t * P:(t + 1) * P,
                                                 db * 512:(db + 1) * 512],
                                    in_=xbs[:, t, :])

    nc.compile()
    return nc


_NC_CACHE = {}


def get_nc():
    if "full" not in _NC_CACHE:
        _NC_CACHE["full"] = build_nc()
    return _NC_CACHE["full"]


def make_in_maps(inputs):
    f32 = np.float32
    fp8 = ml_dtypes.float8_e4m3
    bf = ml_dtypes.bfloat16
    x = np.asarray(inputs["x"], f32)
    B = x.shape[0]
    Wq = np.asarray(inputs["Wq"], f32)
    Wk = np.asarray(inputs["Wk"], f32)
    Wv = np.asarray(inputs["Wv"], f32)
    bq = np.asarray(inputs["bq"], f32)
    bk = np.asarray(inputs["bk"], f32)
    bv = np.asarray(inputs["bv"], f32)
    g1 = np.asarray(inputs["g1"], f32)
    be1 = np.asarray(inputs["be1"], f32)
    g2 = np.asarray(inputs["g2"], f32)
    be2 = np.asarray(inputs["be2"], f32)
    W1 = np.asarray(inputs["W1"], f32)
    b1 = np.asarray(inputs["b1"], f32)
    W2 = np.asarray(inputs["W2"], f32)
    b2 = np.asarray(inputs["b2"], f32)

    # fold LN1 gains into qkv weights, biases into qkv biases
    wq8 = (g1[None, :, None] * Wq * S_W).astype(fp8)
    wk8 = (g1[None, :, None] * Wk * S_W).astype(fp8)
    wv8 = (g1[None, :, None] * Wv * S_W).astype(fp8)
    bqf = (bq + np.einsum("d,hde->he", be1, Wq)).astype(f32)
    bkf = (bk + np.einsum("d,hde->he", be1, Wk)).astype(f32)
    bvf = ((bv + np.einsum("d,hde->he", be1, Wv)) * (S_H * S_W)).astype(f32)
    # fold LN2 gains into W1
    W1f = (g2[:, None] * W1).astype(bf)
    b1f = (b1 + be2 @ W1).astype(f32)
    W2b = W2.astype(bf)

    def arr_qkv(w):
        # [h, 256k+128i+dp, e] -> [h, dp, k, i, e]
        n = w.shape[0]
        return np.ascontiguousarray(
            w.reshape(n, NDP, 2, P, HD).transpose(0, 3, 1, 2, 4))

    shared = {
        "wq8": arr_qkv(wq8),
        "bqf": np.ascontiguousarray(bqf.T),
        # W1 [d*128+dp, f*128+c] -> [f, dp, d, c]
        "W1": np.ascontiguousarray(
            W1f.reshape(ND, P, NF, P).transpose(2, 1, 0, 3)),
        "b1": np.ascontiguousarray(b1f.reshape(NF, P).T),
        # W2 [f*128+fp, db*512+c] -> [db, f, fp, c]
        "W2": np.ascontiguousarray(
            W2b.reshape(NF, P, 4, 512).transpose(2, 0, 1, 3)),
        "b2": b2,
        "id8_in": np.eye(P, dtype=np.float32).astype(fp8),
        "idb_in": np.eye(P, dtype=np.float32).astype(bf),
    }
    in_maps = []
    for c in range(2 * B):
        b, p = c // 2, c % 2
        rows = np.concatenate([np.arange(g * P, (g + 1) * P)
                               for g in range(p, NT, 2)])
        # maskT[sp, i, tp]: -1e9 where (i-p)*128 + sp > tp
        sp = np.arange(P)[:, None, None]
        i = np.arange(2)[None, :, None]
        tp = np.arange(P)[None, None, :]
        m = np.where((i - p) * P + sp > tp, f32(-1e9), f32(0.0))
        im = dict(shared)
        im["x_ctx"] = x[b]
        im["xq"] = x[b][rows]
        im["wk8"] = arr_qkv(wk8[HL * p:HL * (p + 1)])
        # wv: [a, dp, k, i, q*HD+e] for local head groups of 4
        wvs = wv8[HL * p:HL * (p + 1)]
        im["wv8"] = np.ascontiguousarray(
            wvs.reshape(2, HG, NDP, 2, P, HD)
               .transpose(0, 4, 2, 3, 1, 5).reshape(2, P, NDP, 2, HG * HD))
        im["bkf"] = np.ascontiguousarray(bkf[HL * p:HL * (p + 1)].T)
        im["bvf"] = bvf[HL * p:HL * (p + 1)]
        im["maskT"] = np.ascontiguousarray(m)
        in_maps.append(im)
    return in_maps


def assemble(results, B):
    out = np.zeros((B, T, D), np.float32)
    for c in range(2 * B):
        b, p = c // 2, c % 2
        rows = np.concatenate([np.arange(g * P, (g + 1) * P)
                               for g in range(p, NT, 2)])
        out[b][rows] = results[c]["out"]
    return out


def run(inputs, trace=False, **kw):
    nc = get_nc()
    in_maps = make_in_maps(inputs)
    res = bass_utils.run_bass_kernel_spmd(
        nc, in_maps, core_ids=list(range(len(in_maps))), trace=trace, **kw)
    B = np.asarray(inputs["x"]).shape[0]
    return assemble(res.results, B), res


def kernel(**inputs):
    out, _ = run(inputs)
    return out
